# revision 1
# baseline (speedup 1.0000x reference)
"""Trainium2 Bass kernel for nn_BertL2PredictionHead: out = -||x - emb||_2 + bias.

out[b,s,v] = bias[v] - sqrt(max(||x_bs||^2 + ||emb_v||^2 - 2 x_bs.emb_v, 0))
for x (16,128,128) f32, emb (20001,128) f32, bias (1,1,20001) f32.

Sharding: vocab dimension split across 8 NeuronCores (tensor parallel over V),
x replicated. Per core, for each 128-row tile of the 2048x2502 slice:
  psum = (-2 x^T)^T @ embT          f32r (TF32) matmul, 1 cyc/col, same
                                    stationary weights for all 5 chunks
  o    = psum + esq_bcast           DVE tensor_tensor, adds ||emb_v||^2 (fp32)
  o    = Sqrt(o + ||x_m||^2)        one wide ACT per m-tile, per-partition bias
  DMA o -> out slice                one contiguous 1.28 MB store per m-tile
The final negation and the bias add are fused into the host-side gather copy
(np.negative / np.subtract with out=), which costs nothing beyond the copy.
"""
import sys

sys.path.insert(0, "/opt/trn_rl_repo")

import numpy as np
from contextlib import ExitStack

import concourse.bass as bass  # noqa: F401
import concourse.tile as tile
from concourse import bacc, mybir
from concourse.bass_utils import run_bass_kernel_spmd

F32 = mybir.dt.float32
F32R = mybir.dt.float32r

NCORES = 8
B, S, H, V = 16, 128, 128, 20001
BS = B * S                      # 2048 rows
MT = BS // 128                  # 16 m-tiles of 128 rows
VS = 2502                       # vocab slice per core (even: fp32r needs even free dims)
VPAD = VS * NCORES              # 20016
# psum tiles per m-tile: two [128,1024] (2 banks each) + one [128,454]
PW = 1024
TAIL = VS - 2 * PW              # 454


def _tf32(a: np.ndarray) -> np.ndarray:
    """Round fp32 to TF32 (10-bit mantissa, round-to-nearest-even)."""
    u = a.view(np.uint32).astype(np.uint64)
    lsb = (u >> 13) & 1
    u2 = (u + 0x0FFF + lsb) & 0xFFFFFFFF
    return (u2 & ~np.uint64(0x1FFF)).astype(np.uint32).view(np.float32)


_PROG = None  # (nc,) compiled once per process

# build-time options (set before first _build; for perf sweeps)
CFG = {
    "esqb_host": True,    # True: esq arrives pre-broadcast as [128,VS] input
    "t0_fine": True,      # True: 512-wide chunks for m-tile 0
}


def _build():
    global _PROG
    if _PROG is not None:
        return _PROG

    nc = bacc.Bacc("TRN2", target_bir_lowering=False, debug=False)
    esqb_host = CFG["esqb_host"]

    xT2_d = nc.dram_tensor("xT2", [H, BS], F32R, kind="ExternalInput").ap()
    embT_d = nc.dram_tensor("embT", [H, VS], F32R, kind="ExternalInput").ap()
    if esqb_host:
        esq_d = nc.dram_tensor("esqb", [128, VS], F32, kind="ExternalInput").ap()
    else:
        esq_d = nc.dram_tensor("esq", [1, VS], F32, kind="ExternalInput").ap()
    xsqc_d = nc.dram_tensor("xsqc", [128, MT], F32, kind="ExternalInput").ap()
    out_d = nc.dram_tensor("out", [BS, VS], F32, kind="ExternalOutput").ap()

    with tile.TileContext(nc) as tc, ExitStack() as ctx:
        const = ctx.enter_context(tc.tile_pool(name="const", bufs=1))
        opool = ctx.enter_context(tc.tile_pool(name="opool", bufs=4))
        psum = ctx.enter_context(tc.tile_pool(name="psum", bufs=1, space="PSUM"))

        # Staged inputs, split so the first matmuls gate on ~320KB only
        # (subtile deps track per-slice DMA completion). Two HWDGE rings:
        # sync carries x/emb, scalar carries xsqc + the esq broadcast.
        xt_s = const.tile([H, BS], F32R)
        emb_s = const.tile([H, VS], F32R)
        # head pieces on the sync ring (shared with output stores);
        # the bulky rests go to the gpsimd SWDGE ring so queued store
        # transfers are never stuck behind input transfers in the FIFO.
        nc.sync.dma_start(out=emb_s[:, 0:512], in_=embT_d[:, 0:512])
        nc.sync.dma_start(out=xt_s[:, 0:128], in_=xT2_d[:, 0:128])
        nc.sync.dma_start(out=emb_s[:, 512:PW], in_=embT_d[:, 512:PW])
        # esq broadcast to 128 partitions: either shipped pre-broadcast from
        # the host (scalar HWDGE ring, full-rate loads, +1.26MB HBM reads) or
        # replicated on-device via gpsimd SWDGE DMAs (only 10KB of HBM reads,
        # but the replicate itself is descriptor-bound and slower).
        esqb = const.tile([128, VS], F32)
        xsqc_s = const.tile([128, MT], F32)
        if esqb_host:
            # t0-critical esq pieces ride the sync ring (idle between the
            # input heads and the first store); bulk rests on gpsimd are
            # ordered by first-use time.
            nc.sync.dma_start(out=esqb[:, 0:512], in_=esq_d[:, 0:512])
            nc.sync.dma_start(out=xsqc_s[:], in_=xsqc_d[:])
            nc.gpsimd.dma_start(out=esqb[:, 512:PW], in_=esq_d[:, 512:PW])
            nc.gpsimd.dma_start(out=emb_s[:, PW:VS], in_=embT_d[:, PW:VS])
            nc.gpsimd.dma_start(out=esqb[:, PW:VS], in_=esq_d[:, PW:VS])
            nc.gpsimd.dma_start(out=xt_s[:, 128:BS], in_=xT2_d[:, 128:BS])
        else:
            nc.gpsimd.dma_start(out=esqb[:, 0:512],
                                in_=esq_d[:, 0:512].broadcast_to([128, 512]))
            nc.gpsimd.dma_start(out=xsqc_s[:], in_=xsqc_d[:])
            nc.gpsimd.dma_start(out=esqb[:, 512:PW],
                                in_=esq_d[:, 512:PW].broadcast_to([128, PW - 512]))
            nc.gpsimd.dma_start(out=esqb[:, PW:VS],
                                in_=esq_d[:, PW:VS].broadcast_to([128, VS - PW]))
            nc.gpsimd.dma_start(out=emb_s[:, PW:VS], in_=embT_d[:, PW:VS])
            nc.gpsimd.dma_start(out=xt_s[:, 128:BS], in_=xT2_d[:, 128:BS])

        for t in range(MT):
            o_t = opool.tile([128, VS], F32, tag="o", name=f"o{t}")
            xt = xt_s[:, t * 128:(t + 1) * 128]
            pws = []
            for g in range(2):
                pw = psum.tile([128, PW], F32, tag="pw", bufs=3, name=f"pw{t}_{g}")
                for h in range(2):
                    c0 = g * PW + h * 512
                    nc.tensor.matmul(pw[:, h * 512:(h + 1) * 512], xt,
                                     emb_s[:, c0:c0 + 512], start=True, stop=True)
                pws.append(pw)
            pt = psum.tile([128, TAIL], F32, tag="pt", bufs=2, name=f"pt{t}")
            nc.tensor.matmul(pt[:], xt, emb_s[:, 2 * PW:VS], start=True, stop=True)

            def tt(c0, c1):
                # psum source: pws[0] covers [0,PW), pws[1] [PW,2PW), pt tail
                if c1 <= PW:
                    src_ = pws[0][:, c0:c1]
                elif c0 >= 2 * PW:
                    src_ = pt[:, c0 - 2 * PW:c1 - 2 * PW]
                else:
                    src_ = pws[1][:, c0 - PW:c1 - PW]
                nc.vector.tensor_add(o_t[:, c0:c1], src_, esqb[:, c0:c1])

            rows = out_d[t * 128:(t + 1) * 128, :]
            if t == 0 and CFG["t0_fine"]:
                plan = ((0, 512), (512, PW), (PW, PW + 512), (PW + 512, 2 * PW),
                        (2 * PW, VS))
            elif t == 0 or t in (1, MT - 1):
                plan = ((0, PW), (PW, 2 * PW), (2 * PW, VS))
            else:
                plan = None

            if plan is not None:
                for (c0, c1) in plan:
                    tt(c0, c1)
                    nc.scalar.activation(o_t[:, c0:c1], o_t[:, c0:c1],
                                         mybir.ActivationFunctionType.Sqrt,
                                         bias=xsqc_s[:, t:t + 1], scale=1.0)
                    nc.sync.dma_start(out=rows[:, c0:c1], in_=o_t[:, c0:c1])
            else:
                for (c0, c1) in ((0, PW), (PW, 2 * PW), (2 * PW, VS)):
                    tt(c0, c1)
                nc.scalar.activation(o_t[:], o_t[:],
                                     mybir.ActivationFunctionType.Sqrt,
                                     bias=xsqc_s[:, t:t + 1], scale=1.0)
                nc.sync.dma_start(out=rows, in_=o_t[:])

    nc.compile()
    _PROG = (nc,)
    return _PROG


def _prep_in_maps(x: np.ndarray, emb: np.ndarray):
    X = np.asarray(x, dtype=np.float32).reshape(BS, H)
    xT2 = _tf32(np.ascontiguousarray(X.T) * np.float32(-2.0))
    xsq = (X.astype(np.float64) ** 2).sum(axis=1).astype(np.float32)
    xsqc = np.ascontiguousarray(xsq.reshape(MT, 128).T)   # [128, MT]

    embp = np.zeros((VPAD, H), dtype=np.float32)
    embp[:V] = np.asarray(emb, dtype=np.float32)
    embT = _tf32(np.ascontiguousarray(embp.T))            # [H, VPAD]
    esq = (embp.astype(np.float64) ** 2).sum(axis=1).astype(np.float32)

    maps = []
    for c in range(NCORES):
        lo = c * VS
        maps.append({
            "xT2": xT2,
            "embT": np.ascontiguousarray(embT[:, lo:lo + VS]),
            "esqb" if CFG["esqb_host"] else "esq":
                np.ascontiguousarray(np.broadcast_to(esq[lo:lo + VS], (128, VS)))
                if CFG["esqb_host"] else
                np.ascontiguousarray(esq[lo:lo + VS].reshape(1, VS)),
            "xsqc": xsqc,
        })
    return maps


_FAST = None  # cached (jitted_fn, in_names, out_names, out_avals, zeros_fn)


def _run_fast(in_maps):
    """Cached-jit execution path: same lowering as bass2jax.run_bass_via_pjrt
    but the jitted callable is built once per process and the donated output
    buffers are created on-device (no 164MB host->device zero upload)."""
    global _FAST
    import jax
    import jax.numpy as jnp
    from jax.sharding import Mesh, PartitionSpec, NamedSharding
    from jax.experimental.shard_map import shard_map
    from concourse import bass2jax, mybir as _mybir

    (nc,) = _build()
    if _FAST is None:
        bass2jax.install_neuronx_cc_hook()
        pname = nc.partition_id_tensor.name if nc.partition_id_tensor else None
        in_names, out_names, out_avals = [], [], []
        for alloc in nc.m.functions[0].allocations:
            if not isinstance(alloc, _mybir.MemoryLocationSet):
                continue
            name = alloc.memorylocations[0].name
            if alloc.kind == "ExternalInput":
                if name != pname:
                    in_names.append(name)
            elif alloc.kind == "ExternalOutput":
                out_names.append(name)
                out_avals.append(jax.core.ShapedArray(
                    tuple(alloc.tensor_shape), _mybir.dt.np(alloc.dtype)))
        n_params, n_outs = len(in_names), len(out_names)
        all_names = in_names + out_names + ([pname] if pname else [])

        def _body(*args):
            operands = list(args)
            if pname is not None:
                operands.append(bass2jax.partition_id_tensor())
            return tuple(bass2jax._bass_exec_p.bind(
                *operands,
                out_avals=tuple(out_avals),
                in_names=tuple(all_names),
                out_names=tuple(out_names),
                lowering_input_output_aliases=(),
                sim_require_finite=True,
                sim_require_nnan=True,
                nc=nc,
            ))

        devices = jax.devices()[:NCORES]
        mesh = Mesh(np.asarray(devices), ("core",))
        donate = tuple(range(n_params, n_params + n_outs))
        sharded = jax.jit(
            shard_map(_body, mesh=mesh,
                      in_specs=(PartitionSpec("core"),) * (n_params + n_outs),
                      out_specs=(PartitionSpec("core"),) * n_outs,
                      check_rep=False),
            donate_argnums=donate, keep_unused=True)
        shardings = [NamedSharding(mesh, PartitionSpec("core"))] * n_outs
        zero_shapes = [(NCORES * a.shape[0], *a.shape[1:]) for a in out_avals]
        zeros_fn = jax.jit(
            lambda: tuple(jnp.zeros(s, a.dtype)
                          for s, a in zip(zero_shapes, out_avals)),
            out_shardings=tuple(shardings))
        _FAST = (sharded, in_names, out_names, out_avals, zeros_fn)

    sharded, in_names, out_names, out_avals, zeros_fn = _FAST
    concat_in = [np.concatenate([np.asarray(m[name]) for m in in_maps], axis=0)
                 for name in in_names]
    out_arrs = sharded(*concat_in, *zeros_fn())
    results = [dict() for _ in range(NCORES)]
    for i, name in enumerate(out_names):
        rows_per_core = out_avals[i].shape[0]
        for shard in out_arrs[i].addressable_shards:
            core = shard.index[0].start // rows_per_core
            results[core][name] = np.asarray(shard.data)
    return results


def _run_cores(in_maps, trace: bool = False):
    (nc,) = _build()
    if not trace:
        try:
            class _R:
                pass
            r = _R()
            r.results = _run_fast(in_maps)
            return r
        except Exception:
            pass
    return run_bass_kernel_spmd(nc, in_maps, list(range(NCORES)), trace=trace)


def kernel(x: np.ndarray, emb: np.ndarray, bias: np.ndarray) -> np.ndarray:
    in_maps = _prep_in_maps(x, emb)
    res = _run_cores(in_maps)

    bias_np = np.asarray(bias, dtype=np.float32).reshape(-1)
    have_bias = bool(np.any(bias_np))

    # Gather + fused negate (+ bias): out = bias - dist
    out = np.empty((BS, V), dtype=np.float32)
    for c in range(NCORES):
        lo = c * VS
        hi = min(lo + VS, V)
        dist = res.results[c]["out"][:, :hi - lo]
        if have_bias:
            np.subtract(bias_np[lo:hi][None, :], dist, out=out[:, lo:hi])
        else:
            np.negative(dist, out=out[:, lo:hi])
    return out.reshape(B, S, V)

